# revision 20
# baseline (speedup 1.0000x reference)
"""Trainium2 Bass kernel for nn_Diffusion_55087250539164.

8 diffusion steps of a per-pixel-weighted 3x3 depthwise stencil over
x:(4,64,128,128) with weight:(4,576,128,128) (9 taps per channel pixel,
normalized by sum of |taps|).

Sharding: the 4*64=256 (128,128) planes are fully independent -> 32 planes
per core on 8 cores.

Per-core algorithm (all shapes [partition=128, free...]):
  * Each |weight| tap k=(di,dj) is DMA-loaded PRE-SHIFTED by (-(di-1),-(dj-1))
    (cast to bf16 in-flight) into a column-padded tile wsh[128, 8planes, 132]
    so that the per-step products  y_k = w''_k * x  are fully aligned
    elementwise ops (DVE tensor_tensor, bf16 2x mode).
  * Per-pixel normalizer W = sum_k |w_k| is computed with 9 accumulating
    PE matmuls whose stationary operand is a shifted-identity matrix S_di
    (row shift) and whose rhs access pattern is column-shifted (col shift).
    R = 1/W via ACT ln+exp; row-shifted copies of R via PE; taps are
    normalized in place on DVE.
  * Each step: 9 aligned DVE multiplies -> 9 accumulating PE shift-matmuls
    into PSUM (row shift = stationary S_di, col shift = rhs AP offset)
    -> ACT evacuates PSUM to the next bf16 x tile (f32 on the last step).
"""

import numpy as np
import ml_dtypes

import concourse.bass as bass
import concourse.mybir as mybir
from concourse.bass_utils import run_bass_kernel_spmd
from concourse.tile import TileContext
from concourse.alu_op_type import AluOpType

AF = mybir.ActivationFunctionType
BF16 = mybir.dt.bfloat16
F32 = mybir.dt.float32

N_CORES = 8
H = 128
W = 128
PG = 8          # planes per DVE group (one big elementwise op)
MMG = 4         # planes per matmul/psum group (512 f32 = one PSUM bank)
WPAD = W + 4    # y/wsh tiles: zero cols 0,1 and 130,131; data at cols 2..129
RPAD = W + 2    # R~ tiles: zero cols 0 and 129; data at cols 1..128

TAPS = [(di, dj) for di in range(3) for dj in range(3)]
GP_TAPS = {2, 6}        # step-product taps computed on GpSimd (rest: DVE)
GP_NORM_TAPS = {1, 3, 5, 7}  # normalize-mult taps on GpSimd
LAST_RESULTS = None  # BassKernelResults of the most recent kernel() call


def make_shift_mats():
    """S_di[k, m] = 1 iff k == m + di - 1  (lhsT for out[m] = y[m + di - 1]).

    Slots 3..5 are circular-wrap versions (used only for the W normalizer
    stencil, where taps shifted off the edge live in the tile's spare
    row): S_wrap_0 adds (k=127 -> m=0), S_wrap_2 adds (k=0 -> m=127).
    """
    S = np.zeros((128, 6, 128), np.float32)
    for k in range(127):
        S[k, 0, k + 1] = 1.0       # di=0: out[m] = y[m-1]
    for k in range(128):
        S[k, 1, k] = 1.0           # di=1: identity
    for k in range(1, 128):
        S[k, 2, k - 1] = 1.0       # di=2: out[m] = y[m+1]
    S[:, 3:6, :] = S[:, 0:3, :]
    S[127, 3, 0] = 1.0             # wrap row -1 -> spare row 127
    S[0, 5, 127] = 1.0             # wrap row 128 -> spare row 0
    return S


def split_multi_waits(nc, max_w=1):
    """The stock neuronxcc walrus in this container rejects instructions
    carrying more than one sync wait ("Too many sync wait commands", seen on
    the Tile kernel-tail Drain). Split extras onto single-wait Drains
    inserted just before, on the same engine (program order preserves the
    wait-before-execute semantics)."""
    n = 0
    for fn in nc.m.functions:
        for blk in fn.blocks:
            out = []
            for ins in blk.instructions:
                si = ins.sync_info
                if si is not None and len(si.on_wait) > max_w:
                    waits = list(si.on_wait)
                    for w in waits[:-max_w]:
                        n += 1
                        out.append(mybir.InstDrain(
                            name=f"splitwait-{n}", opcode="Drain",
                            engine=ins.engine,
                            sync_info=mybir.SyncInfo(on_wait=[w], on_update=[])))
                    si.on_wait = waits[-max_w:]
                out.append(ins)
            blk.instructions = out
    return n


def build_nc(n_groups: int, steps: int):
    """Emit the per-core Bass program for n_groups*PG planes and `steps` steps."""
    nplanes = n_groups * PG
    nc = bass.Bass()

    xin = nc.dram_tensor("xin", [n_groups, H, PG, WPAD], BF16, kind="ExternalInput")
    win = nc.dram_tensor("win", [n_groups, 9, 128, PG, WPAD], BF16,
                         kind="ExternalInput")
    shifts_bf_d = nc.dram_tensor("shifts_bf", [128, 6, 128], BF16, kind="ExternalInput")
    shifts_f_d = nc.dram_tensor("shifts_f", [128, 6, 128], F32, kind="ExternalInput")
    xout = nc.dram_tensor("xout", [n_groups, H, PG, W], F32, kind="ExternalOutput")

    with TileContext(nc) as tc:
        with tc.tile_pool(name="sb", bufs=1) as sb, \
             tc.tile_pool(name="ps", bufs=1, space="PSUM") as ps:

            # ---- persistent tiles ----
            shifts_bf = sb.tile([128, 6, 128], BF16, tag="shifts_bf")
            shifts_f = sb.tile([128, 6, 128], F32, tag="shifts_f")
            nc.sync.dma_start(out=shifts_bf[:], in_=shifts_bf_d[:])
            nc.sync.dma_start(out=shifts_f[:], in_=shifts_f_d[:])

            wsh = [[sb.tile([128, PG, WPAD], BF16, tag=f"wsh{g}_{k}", name=f"wsh{g}_{k}")
                    for k in range(9)] for g in range(n_groups)]
            xt = [[sb.tile([128, PG, WPAD], BF16, tag=f"x{g}_{t}", name=f"x{g}_{t}")
                   for t in range(2)] for g in range(n_groups)]
            fin = [sb.tile([128, PG, W], F32, tag=f"fin{g}", name=f"fin{g}")
                   for g in range(n_groups)]

            NY = 10
            ytiles = [sb.tile([128, PG, WPAD], BF16, tag=f"y{i}", name=f"y{i}") for i in range(NY)]
            # R~ ring: 3 row-variants x 2 groups in flight
            NR = 2
            rt = [[sb.tile([128, PG, RPAD], F32, tag=f"rt{i}_{d}", name=f"rt{i}_{d}")
                   for d in range(3)] for i in range(NR)]
            lnw = [sb.tile([128, PG, W], F32, tag=f"lnw{i}", name=f"lnw{i}") for i in range(NR)]

            acc_ps = [ps.tile([128, MMG, W], F32, tag=f"acc{i}", name=f"acc{i}") for i in range(4)]
            w_ps = [ps.tile([128, MMG, W], F32, tag=f"wps{i}", name=f"wps{i}") for i in range(2)]
            r_ps = [ps.tile([128, MMG, W], F32, tag=f"rps{i}", name=f"rps{i}") for i in range(2)]

            # one-time border zeroing of the padded x tiles (products read
            # the full padded width; borders must stay finite zeros)
            for row in xt:
                for t in row:
                    nc.vector.memset(t[:, :, 0:2], 0.0)
                    nc.vector.memset(t[:, :, W + 2:W + 4], 0.0)

            yi = [0]          # y ring cursor
            acci = [0]        # psum ring cursor

            def setup_group(g):
                g0 = g * PG
                ring = g % NR
                # ---- load host-baked tap images: one fully-contiguous
                # bf16 DMA per tap (alternating the two HWDGE rings), then
                # ACT abs in place ----
                for k in range(9):
                    eng = nc.sync if k % 2 == 0 else nc.scalar
                    eng.dma_start(out=wsh[g][k][:], in_=win[g, k])
                    nc.scalar.activation(wsh[g][k][:], wsh[g][k][:], AF.Abs)
                # ---- load x (host-baked bf16 layout, contiguous) ----
                nc.sync.dma_start(out=xt[g][0][:], in_=xin[g])
                # ---- W = stencil-sum of |w| taps; lnW; R = exp(-lnW) ----
                for h in range(2):
                    wp = w_ps[h]
                    sl = slice(MMG * h, MMG * (h + 1))
                    for k, (di, dj) in enumerate(TAPS):
                        nc.tensor.matmul(
                            wp[:], shifts_bf[:, di + 3, :],
                            wsh[g][k][:, sl, dj + 1:dj + 1 + W],
                            start=(k == 0), stop=(k == 8))
                    nc.scalar.activation(lnw[ring][:, sl, :], wp[:], AF.Ln)
                r1 = rt[ring][1]
                nc.vector.memset(r1[:, :, 0:1], 0.0)
                nc.vector.memset(r1[:, :, W + 1:W + 2], 0.0)
                nc.scalar.activation(r1[:, :, 1:1 + W], lnw[ring][:], AF.Exp,
                                     scale=-1.0)
                # ---- row-shifted copies of R (f32 matmuls) ----
                for d, smat in ((0, 2), (2, 0)):
                    rd = rt[ring][d]
                    nc.vector.memset(rd[:, :, 0:1], 0.0)
                    nc.vector.memset(rd[:, :, W + 1:W + 2], 0.0)
                    for h in range(2):
                        rp = r_ps[h]
                        nc.tensor.matmul(
                            rp[:], shifts_f[:, smat, :],
                            r1[:, MMG * h:MMG * (h + 1), 1:1 + W],
                            start=True, stop=True)
                        nc.scalar.copy(rd[:, MMG * h:MMG * (h + 1), 1:1 + W], rp[:])
                # ---- normalize taps in place: w'' = |w| * R~ ----
                for k, (di, dj) in enumerate(TAPS):
                    eng = nc.gpsimd if k in GP_NORM_TAPS else nc.vector
                    eng.tensor_tensor(
                        wsh[g][k][:, :, 2:2 + W], wsh[g][k][:, :, 2:2 + W],
                        rt[ring][di][:, :, 2 - dj:2 - dj + W], AluOpType.mult)

            def step_group(g, t):
                cur = xt[g][t % 2]
                last = (t == steps - 1)
                ys = []
                for k in range(9):
                    y = ytiles[yi[0] % NY]
                    yi[0] += 1
                    eng = nc.gpsimd if k in GP_TAPS else nc.vector
                    eng.tensor_tensor(y[:], wsh[g][k][:], cur[:],
                                      AluOpType.mult)
                    ys.append(y)
                for h in range(2):
                    ap = acc_ps[acci[0] % 4]
                    acci[0] += 1
                    for k, (di, dj) in enumerate(TAPS):
                        nc.tensor.matmul(
                            ap[:], shifts_bf[:, di, :],
                            ys[k][:, MMG * h:MMG * (h + 1), dj + 1:dj + 1 + W],
                            start=(k == 0), stop=(k == 8))
                    if last:
                        nc.scalar.copy(fin[g][:, MMG * h:MMG * (h + 1), :],
                                       ap[:])
                    else:
                        nxt = xt[g][(t + 1) % 2]
                        nc.scalar.copy(
                            nxt[:, MMG * h:MMG * (h + 1), 2:2 + W], ap[:])

            for g in range(n_groups):
                setup_group(g)
            for t in range(steps):
                for g in range(n_groups):
                    step_group(g, t)
            for g in range(n_groups):
                g0 = g * PG
                nc.sync.dma_start(out=xout[g], in_=fin[g][:])

    return nc


def make_inputs_for_core(x_planes, w_planes):
    """Bake the per-tap SBUF images on the host (pure layout: transpose,
    circular roll, zero padding, bf16 cast) so every device load is one
    fully-contiguous DMA. whost[g,k,q,gg,c]: window c in [dj+1, dj+129)
    holds w[g0+gg, k, (q+di-1)%128, c-dj-1]; other cols zero."""
    n_groups = x_planes.shape[0] // PG
    whost = np.zeros((n_groups, 9, 128, PG, WPAD), np.float32)
    for g in range(n_groups):
        for k, (di, dj) in enumerate(TAPS):
            plane = w_planes[g * PG:(g + 1) * PG, k]          # [PG, 128, 128]
            rolled = np.roll(plane, di - 1, axis=1)           # row circular
            whost[g, k, :, :, dj + 1:dj + 1 + W] = rolled.transpose(1, 0, 2)
    xhost = np.zeros((n_groups, H, PG, WPAD), np.float32)
    xhost[:, :, :, 2:2 + W] = x_planes.reshape(
        n_groups, PG, H, W).transpose(0, 2, 1, 3)
    S = make_shift_mats()
    return {
        "xin": np.ascontiguousarray(xhost).astype(ml_dtypes.bfloat16),
        "win": whost.astype(ml_dtypes.bfloat16),
        "shifts_bf": S.astype(ml_dtypes.bfloat16),
        "shifts_f": S,
    }


def kernel(x, weight, max_step):
    x = np.asarray(x, np.float32)
    weight = np.asarray(weight, np.float32)
    n, c, h, w = x.shape
    steps = int(min(int(max_step), max(h, w)))
    if steps <= 0:
        return x.copy()

    planes_x = x.reshape(n * c, h, w)
    planes_w = weight.reshape(n * c, 9, h, w)
    per_core = (n * c) // N_CORES

    n_groups = per_core // PG
    nc = build_nc(n_groups, steps)
    split_multi_waits(nc)
    in_maps = []
    for i in range(N_CORES):
        s = slice(i * per_core, (i + 1) * per_core)
        in_maps.append(make_inputs_for_core(planes_x[s], planes_w[s]))
    res = run_bass_kernel_spmd(nc, in_maps, core_ids=list(range(N_CORES)))
    global LAST_RESULTS
    LAST_RESULTS = res
    # xout[g, h, gg, w] -> planes [g*PG+gg, h, w]
    outs = [r["xout"].transpose(0, 2, 1, 3).reshape(per_core, h, w)
            for r in res.results]
    out = np.concatenate(outs, axis=0)
    return out.reshape(n, c, h, w).astype(np.float32)


# revision 21
# speedup vs baseline: 1.2501x; 1.2501x over previous
"""Trainium2 Bass kernel for nn_Diffusion_55087250539164.

8 diffusion steps of a per-pixel-weighted 3x3 depthwise stencil over
x:(4,64,128,128) with weight:(4,576,128,128) (9 taps per channel pixel,
normalized by sum of |taps|).

Sharding: the 4*64=256 (128,128) planes are fully independent -> 32 planes
per core on 8 cores.

Per-core algorithm (all shapes [partition=128, free...]):
  * Each |weight| tap k=(di,dj) is DMA-loaded PRE-SHIFTED by (-(di-1),-(dj-1))
    (cast to bf16 in-flight) into a column-padded tile wsh[128, 8planes, 132]
    so that the per-step products  y_k = w''_k * x  are fully aligned
    elementwise ops (DVE tensor_tensor, bf16 2x mode).
  * Per-pixel normalizer W = sum_k |w_k| is computed with 9 accumulating
    PE matmuls whose stationary operand is a shifted-identity matrix S_di
    (row shift) and whose rhs access pattern is column-shifted (col shift).
    R = 1/W via ACT ln+exp; row-shifted copies of R via PE; taps are
    normalized in place on DVE.
  * Each step: 9 aligned DVE multiplies -> 9 accumulating PE shift-matmuls
    into PSUM (row shift = stationary S_di, col shift = rhs AP offset)
    -> ACT evacuates PSUM to the next bf16 x tile (f32 on the last step).
"""

import numpy as np
import ml_dtypes

import concourse.bass as bass
import concourse.mybir as mybir
from concourse.bass_utils import run_bass_kernel_spmd
from concourse.tile import TileContext
from concourse.alu_op_type import AluOpType

AF = mybir.ActivationFunctionType
BF16 = mybir.dt.bfloat16
F32 = mybir.dt.float32

N_CORES = 8
H = 128
W = 128
PG = 8          # planes per DVE group (one big elementwise op)
MMG = 4         # planes per matmul/psum group (512 f32 = one PSUM bank)
WPAD = W + 4    # y/wsh tiles: zero cols 0,1 and 130,131; data at cols 2..129
RPAD = W + 2    # R~ tiles: zero cols 0 and 129; data at cols 1..128

TAPS = [(di, dj) for di in range(3) for dj in range(3)]
GP_TAPS = set()         # step-product taps on GpSimd (empty: port contention hurts DVE)
GP_NORM_TAPS = set()    # normalize-mult taps on GpSimd
LAST_RESULTS = None  # BassKernelResults of the most recent kernel() call


def make_shift_mats():
    """S_di[k, m] = 1 iff k == m + di - 1  (lhsT for out[m] = y[m + di - 1]).

    Slots 3..5 are circular-wrap versions (used only for the W normalizer
    stencil, where taps shifted off the edge live in the tile's spare
    row): S_wrap_0 adds (k=127 -> m=0), S_wrap_2 adds (k=0 -> m=127).
    """
    S = np.zeros((128, 6, 128), np.float32)
    for k in range(127):
        S[k, 0, k + 1] = 1.0       # di=0: out[m] = y[m-1]
    for k in range(128):
        S[k, 1, k] = 1.0           # di=1: identity
    for k in range(1, 128):
        S[k, 2, k - 1] = 1.0       # di=2: out[m] = y[m+1]
    S[:, 3:6, :] = S[:, 0:3, :]
    S[127, 3, 0] = 1.0             # wrap row -1 -> spare row 127
    S[0, 5, 127] = 1.0             # wrap row 128 -> spare row 0
    return S


def split_multi_waits(nc, max_w=1):
    """The stock neuronxcc walrus in this container rejects instructions
    carrying more than one sync wait ("Too many sync wait commands", seen on
    the Tile kernel-tail Drain). Split extras onto single-wait Drains
    inserted just before, on the same engine (program order preserves the
    wait-before-execute semantics)."""
    n = 0
    for fn in nc.m.functions:
        for blk in fn.blocks:
            out = []
            for ins in blk.instructions:
                si = ins.sync_info
                if si is not None and len(si.on_wait) > max_w:
                    waits = list(si.on_wait)
                    for w in waits[:-max_w]:
                        n += 1
                        out.append(mybir.InstDrain(
                            name=f"splitwait-{n}", opcode="Drain",
                            engine=ins.engine,
                            sync_info=mybir.SyncInfo(on_wait=[w], on_update=[])))
                    si.on_wait = waits[-max_w:]
                out.append(ins)
            blk.instructions = out
    return n


def build_nc(n_groups: int, steps: int):
    """Emit the per-core Bass program for n_groups*PG planes and `steps` steps."""
    nplanes = n_groups * PG
    nc = bass.Bass()

    xin = nc.dram_tensor("xin", [n_groups, H, PG, WPAD], BF16, kind="ExternalInput")
    win = nc.dram_tensor("win", [n_groups, 9, 128, PG, WPAD], BF16,
                         kind="ExternalInput")
    shifts_bf_d = nc.dram_tensor("shifts_bf", [128, 6, 128], BF16, kind="ExternalInput")
    shifts_f_d = nc.dram_tensor("shifts_f", [128, 6, 128], F32, kind="ExternalInput")
    xout = nc.dram_tensor("xout", [n_groups, H, PG, W], F32, kind="ExternalOutput")

    with TileContext(nc) as tc:
        with tc.tile_pool(name="sb", bufs=1) as sb, \
             tc.tile_pool(name="ps", bufs=1, space="PSUM") as ps:

            # ---- persistent tiles ----
            shifts_bf = sb.tile([128, 6, 128], BF16, tag="shifts_bf")
            shifts_f = sb.tile([128, 6, 128], F32, tag="shifts_f")
            nc.sync.dma_start(out=shifts_bf[:], in_=shifts_bf_d[:])
            nc.sync.dma_start(out=shifts_f[:], in_=shifts_f_d[:])

            wsh = [[sb.tile([128, PG, WPAD], BF16, tag=f"wsh{g}_{k}", name=f"wsh{g}_{k}")
                    for k in range(9)] for g in range(n_groups)]
            xt = [[sb.tile([128, PG, WPAD], BF16, tag=f"x{g}_{t}", name=f"x{g}_{t}")
                   for t in range(2)] for g in range(n_groups)]
            fin = [sb.tile([128, PG, W], F32, tag=f"fin{g}", name=f"fin{g}")
                   for g in range(n_groups)]

            NY = 10
            ytiles = [sb.tile([128, PG, WPAD], BF16, tag=f"y{i}", name=f"y{i}") for i in range(NY)]
            # R~ ring: 3 row-variants x 2 groups in flight
            NR = 2
            rt = [[sb.tile([128, PG, RPAD], F32, tag=f"rt{i}_{d}", name=f"rt{i}_{d}")
                   for d in range(3)] for i in range(NR)]
            lnw = [sb.tile([128, PG, W], F32, tag=f"lnw{i}", name=f"lnw{i}") for i in range(NR)]

            acc_ps = [ps.tile([128, MMG, W], F32, tag=f"acc{i}", name=f"acc{i}") for i in range(4)]
            w_ps = [ps.tile([128, MMG, W], F32, tag=f"wps{i}", name=f"wps{i}") for i in range(2)]
            r_ps = [ps.tile([128, MMG, W], F32, tag=f"rps{i}", name=f"rps{i}") for i in range(2)]

            # one-time border zeroing of the padded x tiles (products read
            # the full padded width; borders must stay finite zeros)
            for row in xt:
                for t in row:
                    nc.vector.memset(t[:, :, 0:2], 0.0)
                    nc.vector.memset(t[:, :, W + 2:W + 4], 0.0)

            yi = [0]          # y ring cursor
            acci = [0]        # psum ring cursor

            def setup_group(g):
                g0 = g * PG
                ring = g % NR
                # ---- load host-baked tap images: one fully-contiguous
                # bf16 DMA per tap (alternating the two HWDGE rings), then
                # ACT abs in place ----
                for k in range(9):
                    eng = nc.sync if k % 2 == 0 else nc.scalar
                    eng.dma_start(out=wsh[g][k][:], in_=win[g, k])
                    nc.scalar.activation(wsh[g][k][:], wsh[g][k][:], AF.Abs)
                # ---- load x (host-baked bf16 layout, contiguous) ----
                nc.sync.dma_start(out=xt[g][0][:], in_=xin[g])
                # ---- W = stencil-sum of |w| taps; lnW; R = exp(-lnW) ----
                for k, (di, dj) in enumerate(TAPS):
                    for h in range(2):
                        sl = slice(MMG * h, MMG * (h + 1))
                        nc.tensor.matmul(
                            w_ps[h][:], shifts_bf[:, di + 3, :],
                            wsh[g][k][:, sl, dj + 1:dj + 1 + W],
                            start=(k == 0), stop=(k == 8))
                for h in range(2):
                    sl = slice(MMG * h, MMG * (h + 1))
                    nc.scalar.activation(lnw[ring][:, sl, :], w_ps[h][:], AF.Ln)
                r1 = rt[ring][1]
                nc.vector.memset(r1[:, :, 0:1], 0.0)
                nc.vector.memset(r1[:, :, W + 1:W + 2], 0.0)
                nc.scalar.activation(r1[:, :, 1:1 + W], lnw[ring][:], AF.Exp,
                                     scale=-1.0)
                # ---- row-shifted copies of R (f32 matmuls) ----
                for d, smat in ((0, 2), (2, 0)):
                    rd = rt[ring][d]
                    nc.vector.memset(rd[:, :, 0:1], 0.0)
                    nc.vector.memset(rd[:, :, W + 1:W + 2], 0.0)
                    for h in range(2):
                        rp = r_ps[h]
                        nc.tensor.matmul(
                            rp[:], shifts_f[:, smat, :],
                            r1[:, MMG * h:MMG * (h + 1), 1:1 + W],
                            start=True, stop=True)
                        nc.scalar.copy(rd[:, MMG * h:MMG * (h + 1), 1:1 + W], rp[:])
                # ---- normalize taps in place: w'' = |w| * R~ ----
                for k, (di, dj) in enumerate(TAPS):
                    eng = nc.gpsimd if k in GP_NORM_TAPS else nc.vector
                    eng.tensor_tensor(
                        wsh[g][k][:, :, 2:2 + W], wsh[g][k][:, :, 2:2 + W],
                        rt[ring][di][:, :, 2 - dj:2 - dj + W], AluOpType.mult)

            def step_group(g, t):
                cur = xt[g][t % 2]
                last = (t == steps - 1)
                ys = []
                for k in range(9):
                    y = ytiles[yi[0] % NY]
                    yi[0] += 1
                    eng = nc.gpsimd if k in GP_TAPS else nc.vector
                    eng.tensor_tensor(y[:], wsh[g][k][:], cur[:],
                                      AluOpType.mult)
                    ys.append(y)
                aps = [acc_ps[acci[0] % 4], acc_ps[(acci[0] + 1) % 4]]
                acci[0] += 2
                for k, (di, dj) in enumerate(TAPS):
                    for h in range(2):
                        nc.tensor.matmul(
                            aps[h][:], shifts_bf[:, di, :],
                            ys[k][:, MMG * h:MMG * (h + 1), dj + 1:dj + 1 + W],
                            start=(k == 0), stop=(k == 8))
                for h in range(2):
                    if last:
                        nc.scalar.copy(fin[g][:, MMG * h:MMG * (h + 1), :],
                                       aps[h][:])
                    else:
                        nxt = xt[g][(t + 1) % 2]
                        nc.scalar.copy(
                            nxt[:, MMG * h:MMG * (h + 1), 2:2 + W], aps[h][:])

            for g in range(n_groups):
                setup_group(g)
            for t in range(steps):
                for g in range(n_groups):
                    step_group(g, t)
            for g in range(n_groups):
                g0 = g * PG
                nc.sync.dma_start(out=xout[g], in_=fin[g][:])

    return nc


def make_inputs_for_core(x_planes, w_planes):
    """Bake the per-tap SBUF images on the host (pure layout: transpose,
    circular roll, zero padding, bf16 cast) so every device load is one
    fully-contiguous DMA. whost[g,k,q,gg,c]: window c in [dj+1, dj+129)
    holds w[g0+gg, k, (q+di-1)%128, c-dj-1]; other cols zero."""
    n_groups = x_planes.shape[0] // PG
    whost = np.zeros((n_groups, 9, 128, PG, WPAD), np.float32)
    for g in range(n_groups):
        for k, (di, dj) in enumerate(TAPS):
            plane = w_planes[g * PG:(g + 1) * PG, k]          # [PG, 128, 128]
            rolled = np.roll(plane, di - 1, axis=1)           # row circular
            whost[g, k, :, :, dj + 1:dj + 1 + W] = rolled.transpose(1, 0, 2)
    xhost = np.zeros((n_groups, H, PG, WPAD), np.float32)
    xhost[:, :, :, 2:2 + W] = x_planes.reshape(
        n_groups, PG, H, W).transpose(0, 2, 1, 3)
    S = make_shift_mats()
    return {
        "xin": np.ascontiguousarray(xhost).astype(ml_dtypes.bfloat16),
        "win": whost.astype(ml_dtypes.bfloat16),
        "shifts_bf": S.astype(ml_dtypes.bfloat16),
        "shifts_f": S,
    }


def kernel(x, weight, max_step):
    x = np.asarray(x, np.float32)
    weight = np.asarray(weight, np.float32)
    n, c, h, w = x.shape
    steps = int(min(int(max_step), max(h, w)))
    if steps <= 0:
        return x.copy()

    planes_x = x.reshape(n * c, h, w)
    planes_w = weight.reshape(n * c, 9, h, w)
    per_core = (n * c) // N_CORES

    n_groups = per_core // PG
    nc = build_nc(n_groups, steps)
    split_multi_waits(nc)
    in_maps = []
    for i in range(N_CORES):
        s = slice(i * per_core, (i + 1) * per_core)
        in_maps.append(make_inputs_for_core(planes_x[s], planes_w[s]))
    res = run_bass_kernel_spmd(nc, in_maps, core_ids=list(range(N_CORES)))
    global LAST_RESULTS
    LAST_RESULTS = res
    # xout[g, h, gg, w] -> planes [g*PG+gg, h, w]
    outs = [r["xout"].transpose(0, 2, 1, 3).reshape(per_core, h, w)
            for r in res.results]
    out = np.concatenate(outs, axis=0)
    return out.reshape(n, c, h, w).astype(np.float32)
